# revision 45
# baseline (speedup 1.0000x reference)
"""DiT block (adaLN) Trainium2 kernel, 8-core SPMD, no collectives.

Sharding: core c handles batch b = c//2 and query-token half c%2 (1024 q
tokens).  Each core computes K/V for all 2048 tokens of its batch (the
only duplicated work), so cores never communicate.  The host permutes
each core's token columns so its own 1024 tokens come first (softmax is
invariant to key order), and transposes x to feature-major [D, L] so the
device never transposes anything.

v2 highlights (baseline 1.85ms, v1 1.36ms):
- Weights land in SBUF via large contiguous slab DMAs.
- The residual stream is never SBUF-resident: x streams from DRAM for
  LN1 and again for the proj residual; only the post-attention x2 tile
  stays on chip.  The freed 32KB buys 4 ept tiles so the attention PE
  stream can run gapless.
- Attention emits one global PE stream: AV matmuls trail the QK+exp
  pipeline by 2 steps, wrapping across head-pair iterations, so the PE
  never waits on the ACT engine's exp and stays at full p-state clock.
- LayerNorm spreads work across ACT (square+modulate), DVE (x*rstd),
  GPSIMD (+(-mu*rstd)); softmax denominators ride the AV matmul via a
  ones-column in v_aug; proj/fc2/v biases are K=1 ones-row matmuls into
  the PSUM accumulation; gated residuals are one scalar_tensor_tensor.
"""

import os
import sys
from contextlib import ExitStack

os.environ.setdefault("MYCRO_LOCAL_CACHE", "1")
for _p in ("/opt/trn_rl_repo", "/root/.axon_site/_ro/trn_rl_repo"):
    if os.path.isdir(_p) and _p not in sys.path:
        sys.path.insert(0, _p)

import ml_dtypes
import numpy as np

import concourse.bass as bass
import concourse.tile as tile
from concourse import bacc, mybir
from concourse.bass_utils import run_bass_kernel_spmd

B, L, D, H, HD, MLPD = 4, 2048, 1024, 16, 64, 4096
NCORES = 8
LOWN = L // 2          # own query tokens per core
DC = D // 128          # 8 chunks of the model dim
MC = MLPD // 128       # 32 chunks of the mlp dim
LT = 512               # token tile for matmul free dim
NLT_OWN = LOWN // LT   # 2 token tiles (queries)
WTF = 768              # wt block width (f direction)

f32 = mybir.dt.float32
bf16 = mybir.dt.bfloat16
AF = mybir.ActivationFunctionType
ALU = mybir.AluOpType
BF = ml_dtypes.bfloat16


def build_program():
    # Bacc (not plain Bass): its compile() pass legalizes multi-semaphore
    # waits (event semaphores, nop fusion) that walrus can't encode raw.
    nc = bacc.Bacc()

    def _in(name, shape, dtype):
        return nc.declare_dram_parameter(name, shape, dtype, False)[:]

    xfm = _in("xfm", [D, L], f32)
    temb = _in("temb", [128, DC], f32)
    wqkv = _in("wqkv", [D, 3 * D], bf16)
    bq = _in("bq", [128, DC], f32)
    bk = _in("bk", [128, DC], f32)
    wproj = _in("wproj", [D, D], bf16)
    w1 = _in("w1", [D, MLPD], bf16)
    b1 = _in("b1", [128, MC], f32)
    w2 = _in("w2", [MLPD, D], bf16)
    wtb = _in("wtb", [64 * 128, WTF], bf16)   # blocked: [fp 8][dc 8][128, 768]
    bt = _in("bt", [128, 48], f32)
    brow = _in("brow", [1, 3 * D], bf16)      # [bproj | b2 | bv]
    out = nc.declare_dram_parameter("out_fm", [D, LOWN], f32, True)[:]

    dbg = {}
    if os.environ.get("BASSDBG"):
        dbg["tp"] = nc.declare_dram_parameter("tp_dbg", [128, 48], f32,
                                              True)[:]
        dbg["xmod"] = nc.declare_dram_parameter("xmod_dbg", [D, L], bf16,
                                                True)[:]
        dbg["k"] = nc.declare_dram_parameter("k_dbg", [D, L], bf16, True)[:]
        dbg["x2"] = nc.declare_dram_parameter("x2_dbg", [D, LOWN], f32,
                                              True)[:]
        dbg["h2"] = nc.declare_dram_parameter("h2_dbg", [D, LOWN], bf16,
                                              True)[:]

    with tile.TileContext(nc) as tc:
        _emit_kernel(tc, xfm, temb, wqkv, bq, bk, wproj, w1, b1, w2,
                     wtb, bt, brow, out, dbg)
    nc.finalize()  # runs Bacc.compile(): reg alloc + sync legalization
    return nc


def _emit_kernel(tc, xfm, temb, wqkv, bq, bk, wproj, w1, b1, w2,
                 wtb, bt, brow, out, dbg=None):
    nc = tc.nc
    dbg = dbg or {}

    def _dump(key, src):
        if key in dbg:
            ap = dbg[key]
            if len(ap.shape) == 2 and ap.shape[0] == D:
                ap = ap.rearrange("(c p) t -> p c t", p=128)
            nc.sync.dma_start(out=ap, in_=src)

    # ---------------- persistent constants (left side) ----------------
    ones_f32, fr_ones_f32 = tc.tile([128, 1], f32, name="ones_f32")
    nc.vector.memset(ones_f32, 1.0)
    ones_bf, fr_ones_bf = tc.tile([128, 1], bf16, name="ones_bf")
    nc.vector.memset(ones_bf, 1.0)
    # all-ones [128,128] stationary: LN stat sums write every PSUM row (the
    # same sum), keeping the PE array fully active so the HAM clock gate
    # stays at full rate.
    ones128_bf, fr_ones128 = tc.tile([128, 128], bf16, name="ones128_bf")
    nc.vector.memset(ones128_bf, 1.0)
    onesr_bf, fr_onesr = tc.tile([1, LT], bf16, name="onesr_bf")
    nc.vector.memset(onesr_bf, 1.0)
    warm_mv, fr_warm = tc.tile([128, LT], bf16, name="warm_mv")
    nc.vector.memset(warm_mv, 1.0)
    eps_tile, fr_eps = tc.tile([1, 1], f32, name="eps_tile")
    nc.vector.memset(eps_tile, 1e-5)

    bias_sb = {}
    bias_frees = []
    for name, ap, w in (("bq", bq, DC), ("bk", bk, DC), ("b1", b1, MC),
                        ("bt", bt, 48), ("temb", temb, DC)):
        t, fr = tc.tile([128, w], f32, name=f"sb_{name}")
        nc.sync.dma_start(out=t, in_=ap)
        bias_sb[name] = t
        bias_frees.append(fr)
    brow_sb, fr_brow = tc.tile([1, 3 * D], bf16, name="brow_sb")
    nc.sync.dma_start(out=brow_sb, in_=brow)
    bprojr = brow_sb[:, 0:D]
    b2r = brow_sb[:, D:2 * D]
    bvr = brow_sb[:, 2 * D:3 * D]

    # modulation vectors (computed in phase 0, consumed later)
    tp, fr_tp = tc.tile([128, 48], f32, name="tp")
    s_msa, fr_s1 = tc.tile([128, DC], f32, name="s_msa")
    s_mlp, fr_s2 = tc.tile([128, DC], f32, name="s_mlp")
    shift_msa = tp[:, 0:8]
    gate_msa = tp[:, 16:24]
    shift_mlp = tp[:, 24:32]
    gate_mlp = tp[:, 40:48]

    # xmod sits at the bottom of the right-side stack (freed after V).
    xmod, fr_xmod = tc.tile([128, DC, L], bf16, name="xmod", side="right")
    attn_dram = nc.dram_tensor("attn_scratch", [D, LOWN], bf16)[:]

    xr = xfm.rearrange("(c p) t -> p c t", p=128)

    # ================= phase 0: time modulation vector ================
    with ExitStack() as ph:
        sbp = ph.enter_context(tc.tile_pool(name="p0_sb", bufs=1,
                                            side="right"))
        psp = ph.enter_context(tc.tile_pool(name="p0_ps", bufs=1,
                                            space="PSUM"))
        sig = sbp.tile([128, DC], f32, tag="sig", bufs=1, name="sig")
        nc.scalar.activation(sig, bias_sb["temb"], AF.Sigmoid)
        silu_bf = sbp.tile([128, DC], bf16, tag="silu", bufs=1,
                           name="silu_bf")
        nc.vector.tensor_tensor(silu_bf, bias_sb["temb"], sig, ALU.mult)

        ps_tp = psp.tile([128, 48], f32, name="ps_tp")
        nfp = (6 * D) // WTF          # 8 f-blocks
        nfl = WTF // 128              # 6 cols per block
        for fp in range(nfp):
            # all 8 dc chunks of this f-block, so each output column's
            # PSUM accumulation chain completes before the next one starts
            # (start_tensor_calc pends the whole 2KB zero region).
            wt_sb = sbp.tile([128, DC, WTF], bf16, tag="wt", bufs=2,
                             name="wt_sb")
            nc.sync.dma_start(
                out=wt_sb,
                in_=wtb[fp * 1024:(fp + 1) * 1024, :].rearrange(
                    "(c p) t -> p c t", p=128))
            for fl in range(nfl):
                f = fp * nfl + fl
                for dc in range(DC):
                    nc.tensor.matmul(ps_tp[:, f:f + 1],
                                     wt_sb[:, dc, fl * 128:(fl + 1) * 128],
                                     silu_bf[:, dc:dc + 1],
                                     start=(dc == 0), stop=(dc == DC - 1))
        nc.vector.tensor_tensor(tp, ps_tp, bias_sb["bt"], ALU.add)
        nc.vector.tensor_scalar_add(s_msa, tp[:, 8:16], 1.0)
        nc.vector.tensor_scalar_add(s_mlp, tp[:, 32:40], 1.0)
        _dump("tp", tp)

    # ---- LayerNorm-with-modulation helper (one LT-token tile) ----
    def ln_tile(sbp, psp, x_view, out_view, scale_ap, shift_ap):
        """x_view [128, DC, LT] f32; out_view [128, DC, LT] bf16:
        out = ((x - mu) * rstd) * s_d + sh_d."""
        ps_s = psp.tile([128, LT], f32, tag="st_s", bufs=2, name="ps_s")
        ps_q = psp.tile([128, LT], f32, tag="st_q", bufs=2, name="ps_q")
        for dc in range(DC):
            xs = x_view[:, dc, :]
            # bf16 shadow of x: the sum matmul then runs 1 cycle/row
            # instead of 4 (f32 moving data); mean error ~1e-4.
            xbf = sbp.tile([128, LT], bf16, tag="xbf", bufs=3, name="xbf")
            nc.scalar.activation(xbf, xs, AF.Identity)
            nc.tensor.matmul(ps_s, ones128_bf, xbf,
                             start=(dc == 0), stop=(dc == DC - 1))
            sq = sbp.tile([128, LT], bf16, tag="sq", bufs=3, name="sq")
            nc.vector.tensor_tensor(sq, xbf, xbf, ALU.mult)
            nc.tensor.matmul(ps_q, ones128_bf, sq,
                             start=(dc == 0), stop=(dc == DC - 1))
        mean = sbp.tile([1, LT], f32, tag="ln_mean", bufs=2, name="mean")
        var = sbp.tile([1, LT], f32, tag="ln_var", bufs=2, name="var")
        rstd = sbp.tile([1, LT], f32, tag="ln_rstd", bufs=2, name="rstd")
        mua = sbp.tile([1, LT], f32, tag="ln_mua", bufs=2, name="mua")
        nc.vector.tensor_scalar_mul(mean, ps_s[0:1, :], 1.0 / D)
        nc.vector.tensor_scalar_mul(var, ps_q[0:1, :], 1.0 / D)
        nc.vector.tensor_tensor(mua, mean, mean, ALU.mult)    # mean^2
        nc.vector.tensor_tensor(var, var, mua, ALU.subtract)
        nc.scalar.activation(rstd, var, AF.Sqrt, bias=eps_tile, scale=1.0)
        nc.vector.reciprocal(out=rstd, in_=rstd)
        # mua = -(mean * rstd) so the gpsimd apply step is an Add
        nc.vector.tensor_tensor(mua, mean, rstd, ALU.mult)
        nc.vector.tensor_scalar_mul(mua, mua, -1.0)
        a_bc = sbp.tile([128, LT], f32, tag="ln_abc", bufs=2, name="a_bc")
        nc.gpsimd.partition_broadcast(a_bc, rstd)
        m_bc = sbp.tile([128, LT], f32, tag="ln_mbc", bufs=2, name="m_bc")
        nc.gpsimd.partition_broadcast(m_bc, mua)
        for dc in range(DC):
            t = sbp.tile([128, LT], f32, tag="ln_t", bufs=6, name="ln_t")
            nc.vector.tensor_tensor(t, x_view[:, dc, :], a_bc, ALU.mult)
            nc.gpsimd.tensor_tensor(t, t, m_bc, ALU.add)
            nc.scalar.activation(out_view[:, dc, :], t, AF.Identity,
                                 bias=shift_ap[:, dc:dc + 1],
                                 scale=scale_ap[:, dc:dc + 1])

    # ================= phase 1: LN1 + modulate (x streamed) ================
    with ExitStack() as ph:
        sbp = ph.enter_context(tc.tile_pool(name="p1_sb", bufs=2,
                                            side="right"))
        psp = ph.enter_context(tc.tile_pool(name="p1_ps", bufs=1,
                                            space="PSUM"))
        for st in range(L // LT):
            xo = sbp.tile([128, DC, LT], f32, tag="xst", bufs=3, name="xo")
            nc.sync.dma_start(
                out=xo, in_=xr[:, :, st * LT:(st + 1) * LT])
            ln_tile(sbp, psp, xo,
                    xmod[:, :, st * LT:(st + 1) * LT], s_msa, shift_msa)

    # ------ left-side attention persistents (freed after attention) ------
    # Zero-padded Q copies: qz0 holds even heads' features in partitions
    # 0:64 (rest zero), qz1 odd heads' in 64:128.  QK then contracts over
    # the full 128 partitions -- same cycle count, but the PE array stays
    # fully active so the HAM clock gate holds K=8 (2.4 GHz).
    qz0, fr_qz0 = tc.tile([128, DC, LOWN], bf16, name="qz0")
    qz1, fr_qz1 = tc.tile([128, DC, LOWN], bf16, name="qz1")
    nc.gpsimd.memset(qz0[64:128, :, :], 0.0)
    nc.gpsimd.memset(qz1[0:64, :, :], 0.0)
    k_sb, fr_k = tc.tile([128, DC, L], bf16, name="k_sb")
    # v flattened to [token-part, token-chunk, 16*65+63]: head h occupies
    # cols [h*65, h*65+64), col h*65+64 holds ones (softmax denominator
    # rides the AV matmul), and the 63-col tail pad lets every AV
    # stationary read a full 128 columns (rows 65+ of the result are
    # ignored junk from the next head).
    VW = H * (HD + 1) + 63
    v_aug, fr_v = tc.tile([128, L // 128, VW], bf16, name="v_aug")

    # ================= phase 2a: Q then K (weight-stationary) ============
    with ExitStack() as ph:
        sbp = ph.enter_context(tc.tile_pool(name="p2_sb", bufs=1,
                                            side="right"))
        psp = ph.enter_context(tc.tile_pool(name="p2_ps", bufs=1,
                                            space="PSUM"))
        for part in range(2):   # 0: Q over own tokens, 1: K over all
            w_sb = sbp.tile([128, DC, D], bf16, tag="wqk", bufs=1,
                            name="w_sb")
            for dc in range(DC):
                nc.sync.dma_start(
                    out=w_sb[:, dc, :],
                    in_=wqkv[dc * 128:(dc + 1) * 128, part * D:(part + 1) * D])
            # lt-outer: each token tile's matmuls depend only on its own
            # LN1 output, so QKV overlaps the tail of phase 1.
            nlt = NLT_OWN if part == 0 else L // LT
            for lt in range(nlt):
                for ft in range(DC):
                    ps = psp.tile([128, LT], f32, tag=f"qk{ft % 2}", bufs=2,
                                  name="ps_qk")
                    for dc in range(DC):
                        nc.tensor.matmul(
                            ps, w_sb[:, dc, ft * 128:(ft + 1) * 128],
                            xmod[:, dc, lt * LT:(lt + 1) * LT],
                            start=(dc == 0), stop=(dc == DC - 1))
                    if part == 0:
                        nc.scalar.activation(
                            qz0[0:64, ft, lt * LT:(lt + 1) * LT], ps[0:64, :],
                            AF.Identity,
                            bias=bias_sb["bq"][0:64, ft:ft + 1])
                        nc.scalar.activation(
                            qz1[64:128, ft, lt * LT:(lt + 1) * LT],
                            ps[64:128, :], AF.Identity,
                            bias=bias_sb["bq"][64:128, ft:ft + 1])
                    else:
                        nc.scalar.activation(
                            k_sb[:, ft, lt * LT:(lt + 1) * LT], ps,
                            AF.Identity, bias=bias_sb["bk"][:, ft:ft + 1])

    # ================= phase 2b: V (x-stationary, token-major) ============
    v_hd = v_aug[:, :, 0:H * (HD + 1)].rearrange("p c (h e) -> p c h e",
                                                 e=HD + 1)
    nc.vector.memset(v_hd[:, :, :, HD:], 1.0)
    nc.vector.memset(v_aug[:, :, H * (HD + 1):], 0.0)
    with ExitStack() as ph:
        sbp = ph.enter_context(tc.tile_pool(name="p2v_sb", bufs=1,
                                            side="right"))
        psp = ph.enter_context(tc.tile_pool(name="p2v_ps", bufs=1,
                                            space="PSUM"))
        wv_sb = sbp.tile([128, DC, D], bf16, tag="wv", bufs=1, name="wv_sb")
        for dc in range(DC):
            nc.sync.dma_start(
                out=wv_sb[:, dc, :],
                in_=wqkv[dc * 128:(dc + 1) * 128, 2 * D:3 * D])
        for tcn in range(L // 128):
            psv = psp.tile([128, 2, LT], f32, tag="v", bufs=2, name="ps_v")
            for vs in range(2):
                # bias row: out[t, f] += 1 * bv[f]
                nc.tensor.matmul(psv[:, vs, :], onesr_bf[:, 0:128],
                                 bvr[:, vs * LT:(vs + 1) * LT],
                                 start=True, stop=False)
            for dc in range(DC):
                for vs in range(2):
                    nc.tensor.matmul(
                        psv[:, vs, :],
                        xmod[:, dc, tcn * 128:(tcn + 1) * 128],
                        wv_sb[:, dc, vs * LT:(vs + 1) * LT],
                        start=False, stop=(dc == DC - 1))
            for vs in range(2):
                nc.vector.tensor_copy(
                    out=v_hd[:, tcn, vs * 8:(vs + 1) * 8, :HD],
                    in_=psv[:, vs, :])
        # keep-warm filler: the PE HAM clock gate re-throttles to 1.2GHz
        # after a few us idle, and the QKV->attention transition (pool
        # allocs, exp table load) would otherwise leave such a gap.
        warm = psp.tile([128, LT], f32, tag="warm", bufs=1, name="warm")
        for _ in range(24):
            nc.tensor.matmul(warm, k_sb[:, 0, 0:128], qz0[:, 0, 0:LT],
                             start=True, stop=True)
    _dump("xmod", xmod)
    _dump("k", k_sb)
    fr_xmod()

    # ================= phase 3: attention ================
    # One global PE stream: AV matmuls trail QK+exp by AVLAG steps and wrap
    # across head-pair iterations, so the PE never stalls on the ACT
    # engine's exp and holds its high p-state clock.
    AVLAG = 4
    NIT = NLT_OWN * (H // 2)       # 16 iterations
    NMG = L // 256                 # 8 QK steps per iteration
    with ExitStack() as ph:
        sbp = ph.enter_context(tc.tile_pool(name="p3_sb", bufs=1,
                                            side="right"))
        psp = ph.enter_context(tc.tile_pool(name="p3_ps", bufs=1,
                                            space="PSUM"))
        ept = {}     # iteration -> [tile h0, tile h1]
        ps_av = {}   # iteration -> [psum h0, psum h1]

        def qk_step(n, mg):
            lt, hc = divmod(n, H // 2)
            lts = slice(lt * LT, (lt + 1) * LT)
            if mg == 0:
                ept[n] = [sbp.tile([128, L // 128, LT], bf16,
                                   tag=f"ept{i}_{n % 2}", bufs=1,
                                   name=f"ept{i}") for i in range(2)]
            ps0 = psp.tile([128, 2, LT], f32, tag="sc0", bufs=2, name="ps0")
            ps1 = psp.tile([128, 2, LT], f32, tag="sc1", bufs=1, name="ps1")
            for j in range(2):
                ms = slice((mg * 2 + j) * 128, (mg * 2 + j + 1) * 128)
                # full-width contraction: the zero half of qz contributes
                # nothing but keeps all 128 PE rows active.
                nc.tensor.matmul(ps0[:, j, :], k_sb[:, hc, ms],
                                 qz0[:, hc, lts], start=True, stop=True)
                nc.tensor.matmul(ps1[:, j, :], k_sb[:, hc, ms],
                                 qz1[:, hc, lts], start=True, stop=True)
            # exp((q.k) / 8): the 1/8 rides the activation scale
            nc.scalar.activation(ept[n][0][:, mg * 2:mg * 2 + 2, :], ps0,
                                 AF.Exp, scale=0.125)
            nc.scalar.activation(ept[n][1][:, mg * 2:mg * 2 + 2, :], ps1,
                                 AF.Exp, scale=0.125)

        def av_step(n, a):
            lt, hc = divmod(n, H // 2)
            lts = slice(lt * LT, (lt + 1) * LT)
            if a == 0:
                ps_av[n] = [psp.tile([128, LT], f32, tag=f"av{i}",
                                     bufs=1, name=f"ps_av{i}")
                            for i in range(2)]
            for i in range(2):
                h = 2 * hc + i
                for mcn in (2 * a, 2 * a + 1):
                    nc.tensor.matmul(
                        ps_av[n][i],
                        v_aug[:, mcn, h * (HD + 1):h * (HD + 1) + 128],
                        ept[n][i][:, mcn, :],
                        start=(mcn == 0), stop=(mcn == L // 128 - 1))
            if a != NMG - 1:
                return
            for i in range(2):
                h = 2 * hc + i
                pa = ps_av[n][i]
                # one quick copy frees the PSUM accumulator for the next
                # iteration; the normalize chain reads the SBUF copy.
                av_s = sbp.tile([HD + 1, LT], f32, tag=f"avs{i}", bufs=2,
                                name="av_s")
                nc.vector.tensor_copy(out=av_s, in_=pa[:HD + 1, :])
                rcp = sbp.tile([1, LT], f32, tag="rcp", bufs=2, name="rcp")
                nc.vector.reciprocal(out=rcp, in_=av_s[HD:HD + 1, :])
                rcp_bc = sbp.tile([64, LT], f32, tag="rcpb", bufs=2,
                                  name="rcp_bc")
                nc.gpsimd.partition_broadcast(rcp_bc, rcp)
                at = sbp.tile([64, LT], bf16, tag=f"at{i}", bufs=2,
                              name="at")
                nc.vector.tensor_tensor(at, av_s[:HD, :], rcp_bc, ALU.mult)
                nc.sync.dma_start(
                    out=attn_dram[h * HD:(h + 1) * HD, lts], in_=at)
            del ept[n], ps_av[n]

        for g in range(NIT * NMG + AVLAG):
            if g < NIT * NMG:
                qk_step(*divmod(g, NMG))
            if g >= AVLAG:
                av_step(*divmod(g - AVLAG, NMG))
    fr_v()
    fr_k()
    fr_qz1()
    fr_qz0()

    # ================= phase 4: proj + residual, LN2 ================
    # x2 = x + gate_msa * (attn @ Wproj + bproj), with x re-streamed from
    # DRAM; x2 stays resident for LN2 / fc2 residual / output.
    x2_sb, fr_x2 = tc.tile([128, DC, LOWN], f32, name="x2_sb", side="right")
    h2mod, fr_h2 = tc.tile([128, DC, LOWN], bf16, name="h2mod", side="right")
    with ExitStack() as ph:
        sbp = ph.enter_context(tc.tile_pool(name="p4_sb", bufs=1,
                                            side="right"))
        psp = ph.enter_context(tc.tile_pool(name="p4_ps", bufs=1,
                                            space="PSUM"))
        wpj_sb = sbp.tile([128, DC, D], bf16, tag="wpj", bufs=1,
                          name="wpj_sb")
        for dc in range(DC):
            nc.sync.dma_start(out=wpj_sb[:, dc, :],
                              in_=wproj[dc * 128:(dc + 1) * 128, :])
        # keep-warm filler bridging the wproj/x/attn DMAs so the HAM clock
        # gate doesn't re-throttle the PE across the phase transition.
        warm = psp.tile([128, LT], f32, tag="pj", bufs=2, name="warm")
        for _ in range(48):
            nc.tensor.matmul(warm, ones128_bf, warm_mv,
                             start=True, stop=True)
        ar = attn_dram.rearrange("(c p) t -> p c t", p=128)
        for lt in range(NLT_OWN):
            lts = slice(lt * LT, (lt + 1) * LT)
            xo = sbp.tile([128, DC, LT], f32, tag="xpj", bufs=2, name="xo")
            nc.sync.dma_start(out=xo, in_=xr[:, :, lts])
            attn_t = sbp.tile([128, DC, LT], bf16, tag="at", bufs=2,
                              name="attn_t")
            nc.sync.dma_start(out=attn_t, in_=ar[:, :, lts])
            for ft in range(DC):
                ps = psp.tile([128, LT], f32, tag="pj", bufs=2, name="ps_pj")
                nc.tensor.matmul(ps, bprojr[:, ft * 128:(ft + 1) * 128],
                                 onesr_bf, start=True, stop=False)
                for dc in range(DC):
                    nc.tensor.matmul(
                        ps, wpj_sb[:, dc, ft * 128:(ft + 1) * 128],
                        attn_t[:, dc, :],
                        start=False, stop=(dc == DC - 1))
                nc.vector.scalar_tensor_tensor(
                    out=x2_sb[:, ft, lts], in0=ps,
                    scalar=gate_msa[:, ft:ft + 1], in1=xo[:, ft, :],
                    op0=ALU.mult, op1=ALU.add)
            # LN2 for this lt tile while proj of the next runs on the PE
            ln_tile(sbp, psp, x2_sb[:, :, lts], h2mod[:, :, lts],
                    s_mlp, shift_mlp)
    _dump("x2", x2_sb)
    _dump("h2", h2mod)

    # ================= phase 5: MLP fc1 + gelu ================
    # w1 streams through double-buffered quarter slabs so the reload never
    # stalls the PE mid-phase.
    gelu_sb, fr_gelu = tc.tile([128, MC, LOWN], bf16, name="gelu_sb",
                               side="right")
    with ExitStack() as ph:
        sbp = ph.enter_context(tc.tile_pool(name="p5_sb", bufs=1,
                                            side="right"))
        psp = ph.enter_context(tc.tile_pool(name="p5_ps", bufs=1,
                                            space="PSUM"))
        warm = psp.tile([128, LT], f32, tag="f10", bufs=2, name="warm")
        for _ in range(24):
            nc.tensor.matmul(warm, ones128_bf, warm_mv,
                             start=True, stop=True)
        qw = MLPD // 4
        for quart in range(4):
            w1q = sbp.tile([128, DC, qw], bf16, tag="w1q", bufs=2,
                           name="w1q")
            for dc in range(DC):
                nc.sync.dma_start(
                    out=w1q[:, dc, :],
                    in_=w1[dc * 128:(dc + 1) * 128,
                           quart * qw:(quart + 1) * qw])
            for ftl in range(MC // 4):
                ft = quart * (MC // 4) + ftl
                ps = [psp.tile([128, LT], f32, tag=f"f1{i}", bufs=2,
                               name=f"ps_f1{i}") for i in range(NLT_OWN)]
                for dc in range(DC):
                    wti = w1q[:, dc, ftl * 128:(ftl + 1) * 128]
                    for i in range(NLT_OWN):
                        nc.tensor.matmul(
                            ps[i], wti, h2mod[:, dc, i * LT:(i + 1) * LT],
                            start=(dc == 0), stop=(dc == DC - 1))
                for i in range(NLT_OWN):
                    nc.scalar.activation(
                        gelu_sb[:, ft, i * LT:(i + 1) * LT], ps[i], AF.Gelu,
                        bias=bias_sb["b1"][:, ft:ft + 1])

    # ================= phase 6: fc2 + residual ================
    # One pass per 512-token tile: the 8 output-chunk accumulators use all
    # 8 PSUM banks; w2 slabs stream through a small pool (read twice).
    with ExitStack() as ph:
        sbp = ph.enter_context(tc.tile_pool(name="p6_sb", bufs=1,
                                            side="right"))
        psp = ph.enter_context(tc.tile_pool(name="p6_ps", bufs=1,
                                            space="PSUM"))
        for lt in range(NLT_OWN):
            lts = slice(lt * LT, (lt + 1) * LT)
            ps = [psp.tile([128, LT], f32, tag=f"f2{ft}", bufs=1,
                           name=f"ps_f2{ft}") for ft in range(DC)]
            for ft in range(DC):
                nc.tensor.matmul(ps[ft], b2r[:, ft * 128:(ft + 1) * 128],
                                 onesr_bf, start=True, stop=False)
            for mc in range(MC):
                w2t = sbp.tile([128, D], bf16, tag="w2", bufs=4, name="w2t")
                nc.sync.dma_start(out=w2t,
                                  in_=w2[mc * 128:(mc + 1) * 128, :])
                for ft in range(DC):
                    nc.tensor.matmul(ps[ft],
                                     w2t[:, ft * 128:(ft + 1) * 128],
                                     gelu_sb[:, mc, lts],
                                     start=False, stop=(mc == MC - 1))
            for ft in range(DC):
                # x2 += gate_mlp * (gelu @ W2 + b2), then straight to DRAM
                nc.vector.scalar_tensor_tensor(
                    out=x2_sb[:, ft, lts], in0=ps[ft],
                    scalar=gate_mlp[:, ft:ft + 1], in1=x2_sb[:, ft, lts],
                    op0=ALU.mult, op1=ALU.add)
                nc.sync.dma_start(
                    out=out.rearrange("(c p) t -> p c t", p=128)[:, ft, lts],
                    in_=x2_sb[:, ft, lts])
    fr_gelu()
    fr_h2()
    fr_x2()

    # release remaining persistents in reverse creation order
    fr_s2(); fr_s1(); fr_tp()
    fr_brow()
    for fr in reversed(bias_frees):
        fr()
    fr_eps(); fr_warm(); fr_onesr(); fr_ones128(); fr_ones_bf(); fr_ones_f32()


_PROGRAM_CACHE = {}


def _get_program():
    if "nc" not in _PROGRAM_CACHE:
        _PROGRAM_CACHE["nc"] = build_program()
    return _PROGRAM_CACHE["nc"]


def _fm(v):
    """[D] vector -> feature-major [128, D//128] (partition p, chunk c)."""
    return np.ascontiguousarray(np.asarray(v, np.float32).reshape(-1, 128).T)


def make_in_maps(x, time_emb, Wqkv, bqkv, Wproj, bproj, W1, b1, W2, b2, Wt, bt,
                 g1, be1, g2, be2):
    # g1/be1/g2/be2 are identity layernorm params in this module; verify and
    # fold them away.
    assert np.allclose(g1, 1.0) and np.allclose(g2, 1.0)
    assert np.allclose(be1, 0.0) and np.allclose(be2, 0.0)

    x = np.asarray(x, np.float32)
    # wt blocked [fp 8][dc 8][128, WTF] so every phase-0 DMA is contiguous
    wtb = (np.asarray(Wt, np.float32)
           .reshape(DC, 128, (6 * D) // WTF, WTF)
           .transpose(2, 0, 1, 3)
           .reshape(64 * 128, WTF).astype(BF))
    brow = np.concatenate([np.asarray(bproj, np.float32),
                           np.asarray(b2, np.float32),
                           np.asarray(bqkv[2 * D:], np.float32)])[None, :]
    shared = {
        "wqkv": np.asarray(Wqkv, np.float32).astype(BF),
        "bq": _fm(bqkv[:D]),
        "bk": _fm(bqkv[D:2 * D]),
        "wproj": np.asarray(Wproj, np.float32).astype(BF),
        "w1": np.asarray(W1, np.float32).astype(BF),
        "b1": _fm(b1),
        "w2": np.asarray(W2, np.float32).astype(BF),
        "wtb": np.ascontiguousarray(wtb),
        "bt": _fm(bt),
        "brow": np.ascontiguousarray(brow.astype(BF)),
    }
    in_maps = []
    for c in range(NCORES):
        b, half = c // 2, c % 2
        xb = x[b].T  # [D, L] feature-major
        own = slice(half * LOWN, (half + 1) * LOWN)
        oth = slice((1 - half) * LOWN, (2 - half) * LOWN)
        m = dict(shared)
        m["xfm"] = np.ascontiguousarray(
            np.concatenate([xb[:, own], xb[:, oth]], axis=1))
        m["temb"] = _fm(time_emb[b])
        in_maps.append(m)
    return in_maps


def assemble_output(results):
    outp = np.empty((B, L, D), np.float32)
    for c in range(NCORES):
        b, half = c // 2, c % 2
        outp[b, half * LOWN:(half + 1) * LOWN, :] = results[c]["out_fm"].T
    return outp


def kernel(x, time_emb, Wqkv, bqkv, Wproj, bproj, W1, b1, W2, b2, Wt, bt,
           g1, be1, g2, be2, trace=False, trace_kwargs=None):
    in_maps = make_in_maps(x, time_emb, Wqkv, bqkv, Wproj, bproj, W1, b1,
                           W2, b2, Wt, bt, g1, be1, g2, be2)
    nc = _get_program()
    res = run_bass_kernel_spmd(nc, in_maps, core_ids=list(range(NCORES)),
                               trace=trace, trace_kwargs=trace_kwargs or {})
    kernel.last_results = res
    return assemble_output(res.results)
